# revision 11
# baseline (speedup 1.0000x reference)
"""Distributed GQA causal attention forward on 8 TRN2 NeuronCores.

Problem shapes: residual [B=2, S=2048, D=2048]; W_Q/W_O [32, 64, 2048];
W_K/W_V [8, 64, 2048]; GQA rep=4; causal softmax attention; out [2, 2048, 2048].

Sharding (tensor parallel over heads, following the GQA structure):
  core c owns q-heads [4c, 4c+4) and kv-head c — exactly one GQA group, so
  attention is fully local. Each core computes Q/K/V projections for its
  heads over the full sequence, flash-style causal attention, and a partial
  output projection; partial outputs are summed with a chunked bf16
  ReduceScatter so each core emits a disjoint 512-row shard that the host
  reassembles.

All matmul operands are bf16 (fp32 PSUM accumulation); the scores scale
1/sqrt(64) is folded into W_Q on the host. Softmax skips max-subtraction
(logits are bounded ~|5| for this data distribution) and row-sums come from a
ones-column appended to V. Scores matmuls contract over d_head=64, so head
pairs are packed into PE row groups (0-63 / 64-127) to run concurrently;
K^T is stored duplicated across both partition halves to satisfy the
matmul base-partition constraint.
"""

import sys

for _p in ("/opt/trn_rl_repo", "/root/.axon_site/_ro/trn_rl_repo"):
    if _p not in sys.path:
        sys.path.insert(0, _p)

import numpy as np
from concourse import bacc, mybir, tile
from concourse import bass_utils

N_CORES = 8
B, S, D = 2, 2048, 2048
NH, NKV, DH = 32, 8, 64
NH_LOC = NH // N_CORES  # 4 q-heads per core
SEQ = B * S  # 4096 global rows, b-major
NHL = NH_LOC * DH  # 256 local q-head dim
P = 128
QG = 512  # q-group size (4 tiles of 128)
N_RCHUNK = SEQ // QG  # 8
N_DT = D // P  # 16 d-tiles
N_KT = S // P  # 16 key blocks per batch
N_CHUNK = 4  # ReduceScatter chunks (rows 1024 each)

BF16 = mybir.dt.bfloat16
F32 = mybir.dt.float32
NP_BF16 = mybir.dt.np(BF16)

_compiled = None


def _build():
    nc = bacc.Bacc("TRN2", target_bir_lowering=False, debug=False, num_devices=N_CORES)

    resid_t = nc.dram_tensor("resid_t", [D, SEQ], BF16, kind="ExternalInput")
    wqt = nc.dram_tensor("wqt", [D, NHL], BF16, kind="ExternalInput")
    wkvt = nc.dram_tensor("wkvt", [D, 2 * DH], BF16, kind="ExternalInput")
    wo = nc.dram_tensor("wo", [NHL, D], BF16, kind="ExternalInput")
    mask = nc.dram_tensor("mask", [P, P], BF16, kind="ExternalInput")
    ident = nc.dram_tensor("ident", [P, P], F32, kind="ExternalInput")
    out = nc.dram_tensor("out", [SEQ // N_CORES, D], F32, kind="ExternalOutput")

    rs_in = [
        nc.dram_tensor(f"rs_in{k}", [SEQ // N_CHUNK, D], BF16, kind="Internal")
        for k in range(N_CHUNK)
    ]
    rs_out = [
        nc.dram_tensor(f"rs_out{k}", [P, D], BF16, kind="Internal")
        for k in range(N_CHUNK)
    ]
    rg = [list(range(N_CORES))]
    COPY = mybir.ActivationFunctionType.Copy
    EXP = mybir.ActivationFunctionType.Exp

    with tile.TileContext(nc) as tc:
        with (
            tc.tile_pool(name="persist", bufs=1) as pp,
            tc.tile_pool(name="stream", bufs=3) as sp,
            tc.tile_pool(name="rstream", bufs=8) as rp,
            tc.tile_pool(name="pstream", bufs=4) as xp,
            tc.tile_pool(name="outbuf", bufs=3) as op,
        ):
            # ---- persistent SBUF tensors ----
            qT_sb = [pp.tile([P, SEQ], BF16, name=f"qT{i}") for i in range(2)]
            kT_sb = pp.tile([P, SEQ], BF16, name="kT")  # K^T duplicated in both halves
            v_sb = [pp.tile([P, DH + 1], BF16, name=f"v{rt}") for rt in range(SEQ // P)]
            attn_sb = [pp.tile([P, SEQ], BF16, name=f"attn{i}") for i in range(2)]
            wqt_sb = [pp.tile([P, NHL], BF16, name=f"wqt{i}") for i in range(N_DT)]
            wkvt_sb = [pp.tile([P, 2 * DH], BF16, name=f"wkvt{i}") for i in range(N_DT)]
            wo_sb = [pp.tile([P, D], BF16, name=f"wo{i}") for i in range(2)]
            mask_sb = pp.tile([P, P], BF16, name="mask")
            ident_sb = pp.tile([P, P], F32, name="ident")

            nc.sync.dma_start(mask_sb[:], mask.ap())
            nc.sync.dma_start(ident_sb[:], ident.ap())
            for i in range(N_DT):
                nc.sync.dma_start(wqt_sb[i][:], wqt.ap()[i * P : (i + 1) * P, :])
                nc.sync.dma_start(wkvt_sb[i][:], wkvt.ap()[i * P : (i + 1) * P, :])
            for i in range(2):
                nc.sync.dma_start(wo_sb[i][:], wo.ap()[i * P : (i + 1) * P, :])
            for rt in range(SEQ // P):
                nc.vector.memset(v_sb[rt][:, DH : DH + 1], 1.0)

            # ---- phase A: Q / K / V projections ----
            # residual^T streamed in [128 d, 512 row] tiles via DMA transpose;
            # Q^T accumulated in [128 nh, 512] psum, K^T/V^T in a shared
            # [128, 512] psum (rows 0:64 = K^T, 64:128 = V^T).
            with tc.tile_pool(name="psA", bufs=2, space="PSUM") as psA:
                for rc in range(N_RCHUNK):
                    r0 = rc * QG
                    qp = [psA.tile([P, QG], F32, tag=f"qp{i}", name=f"qp{i}") for i in range(2)]
                    kvp = psA.tile([P, QG], F32, tag="kvp", name="kvp")
                    for dt_ in range(N_DT):
                        rt_tile = rp.tile([P, QG], BF16, tag="residT", name="residT")
                        nc.sync.dma_start(
                            rt_tile[:],
                            resid_t.ap()[dt_ * P : (dt_ + 1) * P, r0 : r0 + QG],
                        )
                        st = dict(start=(dt_ == 0), stop=(dt_ == N_DT - 1))
                        for hb in range(2):
                            nc.tensor.matmul(
                                qp[hb][:],
                                wqt_sb[dt_][:, hb * P : (hb + 1) * P],
                                rt_tile[:],
                                **st,
                            )
                        nc.tensor.matmul(kvp[:], wkvt_sb[dt_][:], rt_tile[:], **st)
                    for hb in range(2):
                        nc.scalar.activation(qT_sb[hb][:, r0 : r0 + QG], qp[hb][:], COPY)
                    nc.scalar.activation(kT_sb[0:DH, r0 : r0 + QG], kvp[0:DH, :], COPY)
                    nc.vector.tensor_copy(kT_sb[DH : 2 * DH, r0 : r0 + QG], kvp[0:DH, :])
                    # V^T -> V via PE transpose (per 128-key tile)
                    vt_tmp = sp.tile([DH, QG], F32, tag="vt_tmp", name="vt_tmp")
                    nc.vector.tensor_copy(vt_tmp[:], kvp[DH : 2 * DH, :])
                    for j in range(QG // P):
                        vtr = psA.tile([P, DH], F32, tag="vtr", name="vtr")
                        nc.tensor.transpose(
                            vtr[:], vt_tmp[:, j * P : (j + 1) * P], ident_sb[0:DH, 0:DH]
                        )
                        nc.vector.tensor_copy(v_sb[rc * 4 + j][:, 0:DH], vtr[:])

            # ---- phases B+C interleaved per ReduceScatter chunk ----
            # chunk k covers global rows [1024k, 1024k+1024) = q-groups {2k, 2k+1}
            # of batch k//2. Head pairs (2i, 2i+1) run in PE row groups 0/64.
            with (
                tc.tile_pool(name="psS", bufs=2, space="PSUM") as psS,
                tc.tile_pool(name="psT", bufs=4, space="PSUM") as psT,
            ):
                last_osb_dma = None
                for k in range(N_CHUNK):
                    b = k // 2
                    for g in (2 * (k % 2), 2 * (k % 2) + 1):
                        # all 4 heads of this q-group together: head pairs hb=0/1
                        # in PE row groups 0/64, interleaved per key block so the
                        # PE fills each pair's exp-wait with the other pair's MMs
                        at = [
                            psT.tile([DH + 1, QG], F32, tag="at", name="at")
                            for _ in range(4)
                        ]
                        for kb in range(4 * g + 4):
                            j = max(0, kb - 4 * g)
                            qoff = b * S + g * QG + j * P
                            n = QG - j * P
                            k0 = b * S + kb * P
                            pts = []
                            for hb in range(2):
                                sc = psS.tile([P, 2, QG], F32, tag="sc", name="sc")
                                for u in range(2):
                                    lo = u * DH
                                    nc.tensor.matmul(
                                        sc[:, u, :n],
                                        kT_sb[lo : lo + DH, k0 : k0 + P],
                                        qT_sb[hb][lo : lo + DH, qoff : qoff + n],
                                        start=True,
                                        stop=True,
                                    )
                                pt = xp.tile([P, 2, QG], BF16, tag="p_sb", name="p_sb")
                                nc.scalar.activation(pt[:, :, :n], sc[:, :, :n], EXP)
                                if kb >= 4 * g:
                                    nc.vector.tensor_tensor(
                                        pt[:, :, 0:P],
                                        pt[:, :, 0:P],
                                        mask_sb[:].unsqueeze(1).broadcast_to([P, 2, P]),
                                        mybir.AluOpType.mult,
                                    )
                                pts.append(pt)
                            for hb in range(2):
                                for u in range(2):
                                    nc.tensor.matmul(
                                        at[2 * hb + u][:, j * P : QG],
                                        v_sb[b * N_KT + kb][:],
                                        pts[hb][:, u, :n],
                                        start=(kb == 0),
                                        stop=(kb == 4 * g + 3),
                                    )
                        for hb in range(2):
                            for u in range(2):
                                hp = u * DH
                                a = at[2 * hb + u]
                                stg = sp.tile([DH, QG], BF16, tag="stg", name="stg")
                                nc.vector.tensor_copy(stg[:], a[0:DH, :])
                                sm = sp.tile([1, QG], F32, tag="sm", name="sm")
                                nc.vector.tensor_copy(sm[:], a[DH : DH + 1, :])
                                recip = sp.tile([1, QG], F32, tag="recip", name="recip")
                                nc.vector.reciprocal_approx_fast(recip[:], sm[:])
                                bc = sp.tile([DH, QG], F32, tag="bc", name="bc")
                                nc.gpsimd.partition_broadcast(bc[:], recip[:])
                                nc.vector.tensor_tensor(
                                    attn_sb[hb][
                                        hp : hp + DH, b * S + g * QG : b * S + (g + 1) * QG
                                    ],
                                    stg[:],
                                    bc[:],
                                    mybir.AluOpType.mult,
                                )
                        # output projection for this q-group's 4 q-tiles
                        for qt4 in range(4):
                            qt = (g % 2) * 4 + qt4
                            grt = k * 8 + qt  # global 128-row tile
                            col0 = grt * P
                            o_sb = op.tile([P, D], BF16, tag="o_sb", name="o_sb")
                            for dti in range(4):
                                ops = psS.tile([P, 2, QG], F32, tag="sc", name="sc")
                                for hb in range(2):
                                    nc.tensor.matmul(
                                        ops[:, 0, :],
                                        attn_sb[hb][:, col0 : col0 + P],
                                        wo_sb[hb][:, dti * 512 : (dti + 1) * 512],
                                        start=(hb == 0),
                                        stop=(hb == 1),
                                    )
                                nc.vector.tensor_copy(
                                    o_sb[:, dti * 512 : (dti + 1) * 512], ops[:, 0, :]
                                )
                            last_osb_dma = nc.sync.dma_start(
                                rs_in[k].ap()[qt * P : (qt + 1) * P, :], o_sb[:]
                            )
                    nc.gpsimd.collective_compute(
                        "ReduceScatter",
                        mybir.AluOpType.add,
                        replica_groups=rg,
                        ins=[rs_in[k].ap().opt()],
                        outs=[rs_out[k].ap().opt()],
                    )

                # ---- readback: bf16 shard -> f32 output ----
                # explicitly ordered after the last chunk's output DMA so the
                # collective-wait doesn't head-of-line block the Sync/DVE queues
                # mid-kernel (engine queues are in-order).
                from concourse.tile_rust import add_dep_helper

                for k in range(N_CHUNK):
                    rb = op.tile([P, D], BF16, tag="rb", name="rb")
                    rb_dma = nc.sync.dma_start(rb[:], rs_out[k].ap())
                    add_dep_helper(
                        rb_dma.ins, last_osb_dma.ins, False, "readback after compute"
                    )
                    rb32 = op.tile([P, D], F32, tag="rb32", name="rb32")
                    nc.vector.tensor_copy(rb32[:], rb[:])
                    nc.sync.dma_start(out.ap()[k * P : (k + 1) * P, :], rb32[:])

    nc.compile()
    return nc


def _get_compiled():
    global _compiled
    if _compiled is None:
        _compiled = _build()
    return _compiled


def kernel(residual, W_Q, W_K, W_V, W_O):
    nc = _get_compiled()

    resid_t = np.ascontiguousarray(residual.reshape(SEQ, D).T.astype(np.float32)).astype(NP_BF16)
    # fold the 1/sqrt(DH) score scale into W_Q
    wq2 = (W_Q.astype(np.float64) / np.sqrt(DH)).reshape(NH * DH, D).astype(np.float32)
    wqt_full = np.ascontiguousarray(wq2.T)  # [D, NH*DH]
    wkt_full = np.ascontiguousarray(W_K.reshape(NKV * DH, D).T)  # [D, NKV*DH]
    wvt_full = np.ascontiguousarray(W_V.reshape(NKV * DH, D).T)
    wo_full = W_O.reshape(NH * DH, D)  # [NH*DH, D]

    mask_np = np.triu(np.ones((P, P), dtype=np.float32)).astype(NP_BF16)  # [k, q]: q>=k
    ident_np = np.eye(P, dtype=np.float32)

    in_maps = []
    for c in range(N_CORES):
        in_maps.append(
            {
                "resid_t": resid_t,
                "wqt": np.ascontiguousarray(
                    wqt_full[:, c * NHL : (c + 1) * NHL]
                ).astype(NP_BF16),
                "wkvt": np.ascontiguousarray(
                    np.concatenate(
                        [
                            wkt_full[:, c * DH : (c + 1) * DH],
                            wvt_full[:, c * DH : (c + 1) * DH],
                        ],
                        axis=1,
                    )
                ).astype(NP_BF16),
                "wo": np.ascontiguousarray(
                    wo_full[c * NHL : (c + 1) * NHL, :]
                ).astype(NP_BF16),
                "mask": mask_np,
                "ident": ident_np,
            }
        )

    res = bass_utils.run_bass_kernel_spmd(
        nc, in_maps, core_ids=list(range(N_CORES))
    )
    kernel.last_results = res

    out_full = np.empty((SEQ, D), dtype=np.float32)
    for c in range(N_CORES):
        shard = res.results[c]["out"]  # [512, D]
        for k in range(N_CHUNK):
            out_full[1024 * k + P * c : 1024 * k + P * (c + 1)] = shard[
                k * P : (k + 1) * P
            ]
    return out_full.reshape(B, S, D)


# revision 12
# speedup vs baseline: 1.1040x; 1.1040x over previous
"""Distributed GQA causal attention forward on 8 TRN2 NeuronCores.

Problem shapes: residual [B=2, S=2048, D=2048]; W_Q/W_O [32, 64, 2048];
W_K/W_V [8, 64, 2048]; GQA rep=4; causal softmax attention; out [2, 2048, 2048].

Sharding (tensor parallel over heads, following the GQA structure):
  core c owns q-heads [4c, 4c+4) and kv-head c — exactly one GQA group, so
  attention is fully local. Each core computes Q/K/V projections for its
  heads over the full sequence, flash-style causal attention, and a partial
  output projection; partial outputs are summed with a chunked bf16
  ReduceScatter so each core emits a disjoint 512-row shard that the host
  reassembles.

All matmul operands are bf16 (fp32 PSUM accumulation); the scores scale
1/sqrt(64) is folded into W_Q on the host. Softmax skips max-subtraction
(logits are bounded ~|5| for this data distribution) and row-sums come from a
ones-column appended to V. Scores matmuls contract over d_head=64, so head
pairs are packed into PE row groups (0-63 / 64-127) to run concurrently;
K^T is stored duplicated across both partition halves to satisfy the
matmul base-partition constraint.
"""

import sys

for _p in ("/opt/trn_rl_repo", "/root/.axon_site/_ro/trn_rl_repo"):
    if _p not in sys.path:
        sys.path.insert(0, _p)

import numpy as np
from concourse import bacc, mybir, tile
from concourse import bass_utils

N_CORES = 8
B, S, D = 2, 2048, 2048
NH, NKV, DH = 32, 8, 64
NH_LOC = NH // N_CORES  # 4 q-heads per core
SEQ = B * S  # 4096 global rows, b-major
NHL = NH_LOC * DH  # 256 local q-head dim
P = 128
QG = 512  # q-group size (4 tiles of 128)
N_RCHUNK = SEQ // QG  # 8
N_DT = D // P  # 16 d-tiles
N_KT = S // P  # 16 key blocks per batch
N_CHUNK = 4  # ReduceScatter chunks (rows 1024 each)

BF16 = mybir.dt.bfloat16
F32 = mybir.dt.float32
NP_BF16 = mybir.dt.np(BF16)

_compiled = None


def _build():
    nc = bacc.Bacc("TRN2", target_bir_lowering=False, debug=False, num_devices=N_CORES)

    resid_t = nc.dram_tensor("resid_t", [D, SEQ], BF16, kind="ExternalInput")
    wqt = nc.dram_tensor("wqt", [D, NHL], BF16, kind="ExternalInput")
    wkvt = nc.dram_tensor("wkvt", [D, 2 * DH], BF16, kind="ExternalInput")
    wo = nc.dram_tensor("wo", [NHL, D], BF16, kind="ExternalInput")
    mask = nc.dram_tensor("mask", [P, P], BF16, kind="ExternalInput")
    ident = nc.dram_tensor("ident", [P, P], F32, kind="ExternalInput")
    out = nc.dram_tensor("out", [SEQ // N_CORES, D], F32, kind="ExternalOutput")

    rs_in = [
        nc.dram_tensor(f"rs_in{k}", [SEQ // N_CHUNK, D], BF16, kind="Internal")
        for k in range(N_CHUNK)
    ]
    rs_out = [
        nc.dram_tensor(f"rs_out{k}", [P, D], BF16, kind="Internal")
        for k in range(N_CHUNK)
    ]
    rg = [list(range(N_CORES))]
    COPY = mybir.ActivationFunctionType.Copy
    EXP = mybir.ActivationFunctionType.Exp

    with tile.TileContext(nc) as tc:
        with (
            tc.tile_pool(name="persist", bufs=1) as pp,
            tc.tile_pool(name="stream", bufs=3) as sp,
            tc.tile_pool(name="rstream", bufs=8) as rp,
            tc.tile_pool(name="pstream", bufs=4) as xp,
            tc.tile_pool(name="outbuf", bufs=3) as op,
        ):
            # ---- persistent SBUF tensors ----
            qT_sb = [pp.tile([P, SEQ], BF16, name=f"qT{i}") for i in range(2)]
            kT_sb = pp.tile([P, SEQ], BF16, name="kT")  # K^T duplicated in both halves
            v_sb = [pp.tile([P, DH + 1], BF16, name=f"v{rt}") for rt in range(SEQ // P)]
            attn_sb = [pp.tile([P, SEQ], BF16, name=f"attn{i}") for i in range(2)]
            wqt_sb = [pp.tile([P, NHL], BF16, name=f"wqt{i}") for i in range(N_DT)]
            wkvt_sb = [pp.tile([P, 2 * DH], BF16, name=f"wkvt{i}") for i in range(N_DT)]
            wo_sb = [pp.tile([P, D], BF16, name=f"wo{i}") for i in range(2)]
            mask_sb = pp.tile([P, P], BF16, name="mask")
            ident_sb = pp.tile([P, P], F32, name="ident")

            nc.sync.dma_start(mask_sb[:], mask.ap())
            nc.sync.dma_start(ident_sb[:], ident.ap())
            for i in range(N_DT):
                nc.sync.dma_start(wqt_sb[i][:], wqt.ap()[i * P : (i + 1) * P, :])
                nc.sync.dma_start(wkvt_sb[i][:], wkvt.ap()[i * P : (i + 1) * P, :])
            for i in range(2):
                nc.sync.dma_start(wo_sb[i][:], wo.ap()[i * P : (i + 1) * P, :])
            for rt in range(SEQ // P):
                nc.vector.memset(v_sb[rt][:, DH : DH + 1], 1.0)

            # ---- phase A: Q / K / V projections ----
            # residual^T streamed in [128 d, 512 row] tiles via DMA transpose;
            # Q^T accumulated in [128 nh, 512] psum, K^T/V^T in a shared
            # [128, 512] psum (rows 0:64 = K^T, 64:128 = V^T).
            with tc.tile_pool(name="psA", bufs=2, space="PSUM") as psA:
                for rc in range(N_RCHUNK):
                    r0 = rc * QG
                    qp = [psA.tile([P, QG], F32, tag=f"qp{i}", name=f"qp{i}") for i in range(2)]
                    kvp = psA.tile([P, QG], F32, tag="kvp", name="kvp")
                    for dt_ in range(N_DT):
                        rt_tile = rp.tile([P, QG], BF16, tag="residT", name="residT")
                        nc.sync.dma_start(
                            rt_tile[:],
                            resid_t.ap()[dt_ * P : (dt_ + 1) * P, r0 : r0 + QG],
                        )
                        st = dict(start=(dt_ == 0), stop=(dt_ == N_DT - 1))
                        for hb in range(2):
                            nc.tensor.matmul(
                                qp[hb][:],
                                wqt_sb[dt_][:, hb * P : (hb + 1) * P],
                                rt_tile[:],
                                **st,
                            )
                        nc.tensor.matmul(kvp[:], wkvt_sb[dt_][:], rt_tile[:], **st)
                    for hb in range(2):
                        nc.scalar.activation(qT_sb[hb][:, r0 : r0 + QG], qp[hb][:], COPY)
                    nc.scalar.activation(kT_sb[0:DH, r0 : r0 + QG], kvp[0:DH, :], COPY)
                    nc.vector.tensor_copy(kT_sb[DH : 2 * DH, r0 : r0 + QG], kvp[0:DH, :])
                    # V^T -> V via PE transpose (per 128-key tile)
                    vt_tmp = sp.tile([DH, QG], F32, tag="vt_tmp", name="vt_tmp")
                    nc.vector.tensor_copy(vt_tmp[:], kvp[DH : 2 * DH, :])
                    for j in range(QG // P):
                        vtr = psA.tile([P, DH], F32, tag="vtr", name="vtr")
                        nc.tensor.transpose(
                            vtr[:], vt_tmp[:, j * P : (j + 1) * P], ident_sb[0:DH, 0:DH]
                        )
                        nc.vector.tensor_copy(v_sb[rc * 4 + j][:, 0:DH], vtr[:])

            # ---- phases B+C interleaved per ReduceScatter chunk ----
            # chunk k covers global rows [1024k, 1024k+1024) = q-groups {2k, 2k+1}
            # of batch k//2. Head pairs (2i, 2i+1) run in PE row groups 0/64.
            with (
                tc.tile_pool(name="psS", bufs=2, space="PSUM") as psS,
                tc.tile_pool(name="psT", bufs=4, space="PSUM") as psT,
            ):
                last_osb_dma = None
                for k in range(N_CHUNK):
                    b = k // 2
                    for g in (2 * (k % 2), 2 * (k % 2) + 1):
                        # all 4 heads of this q-group together: head pairs hb=0/1
                        # in PE row groups 0/64, interleaved per key block so the
                        # PE fills each pair's exp-wait with the other pair's MMs
                        at = [
                            psT.tile([DH + 1, QG], F32, tag="at", name="at")
                            for _ in range(4)
                        ]
                        for kb in range(4 * g + 4):
                            j = max(0, kb - 4 * g)
                            qoff = b * S + g * QG + j * P
                            n = QG - j * P
                            k0 = b * S + kb * P
                            pts = []
                            for hb in range(2):
                                sc = psS.tile([P, 2, QG], F32, tag="sc", name="sc")
                                for u in range(2):
                                    lo = u * DH
                                    nc.tensor.matmul(
                                        sc[:, u, :n],
                                        kT_sb[lo : lo + DH, k0 : k0 + P],
                                        qT_sb[hb][lo : lo + DH, qoff : qoff + n],
                                        start=True,
                                        stop=True,
                                    )
                                pt = xp.tile([P, 2, QG], BF16, tag="p_sb", name="p_sb")
                                nc.scalar.activation(pt[:, :, :n], sc[:, :, :n], EXP)
                                if kb >= 4 * g:
                                    nc.vector.tensor_tensor(
                                        pt[:, :, 0:P],
                                        pt[:, :, 0:P],
                                        mask_sb[:].unsqueeze(1).broadcast_to([P, 2, P]),
                                        mybir.AluOpType.mult,
                                    )
                                pts.append(pt)
                            for hb in range(2):
                                for u in range(2):
                                    nc.tensor.matmul(
                                        at[2 * hb + u][:, j * P : QG],
                                        v_sb[b * N_KT + kb][:],
                                        pts[hb][:, u, :n],
                                        start=(kb == 0),
                                        stop=(kb == 4 * g + 3),
                                    )
                        for hb in range(2):
                            for u in range(2):
                                hp = u * DH
                                a = at[2 * hb + u]
                                stg = sp.tile([DH, QG], BF16, tag="stg", name="stg")
                                nc.vector.tensor_copy(stg[:], a[0:DH, :])
                                sm = sp.tile([1, QG], F32, tag="sm", name="sm")
                                nc.vector.tensor_copy(sm[:], a[DH : DH + 1, :])
                                recip = sp.tile([1, QG], F32, tag="recip", name="recip")
                                nc.vector.reciprocal_approx_fast(recip[:], sm[:])
                                bc = sp.tile([DH, QG], F32, tag="bc", name="bc")
                                nc.gpsimd.partition_broadcast(bc[:], recip[:])
                                nc.vector.tensor_tensor(
                                    attn_sb[hb][
                                        hp : hp + DH, b * S + g * QG : b * S + (g + 1) * QG
                                    ],
                                    stg[:],
                                    bc[:],
                                    mybir.AluOpType.mult,
                                )
                    # output projection for this chunk's 8 q-tiles
                    for qt in range(8):
                        grt = k * 8 + qt  # global 128-row tile
                        col0 = grt * P
                        o_sb = op.tile([P, D], BF16, tag="o_sb", name="o_sb")
                        for dti in range(4):
                            ops = psS.tile([P, 2, QG], F32, tag="sc", name="sc")
                            for hb in range(2):
                                nc.tensor.matmul(
                                    ops[:, 0, :],
                                    attn_sb[hb][:, col0 : col0 + P],
                                    wo_sb[hb][:, dti * 512 : (dti + 1) * 512],
                                    start=(hb == 0),
                                    stop=(hb == 1),
                                )
                            nc.vector.tensor_copy(
                                o_sb[:, dti * 512 : (dti + 1) * 512], ops[:, 0, :]
                            )
                        last_osb_dma = nc.sync.dma_start(
                            rs_in[k].ap()[qt * P : (qt + 1) * P, :], o_sb[:]
                        )
                    nc.gpsimd.collective_compute(
                        "ReduceScatter",
                        mybir.AluOpType.add,
                        replica_groups=rg,
                        ins=[rs_in[k].ap().opt()],
                        outs=[rs_out[k].ap().opt()],
                    )

                # ---- readback: bf16 shard -> f32 output ----
                # explicitly ordered after the last chunk's output DMA so the
                # collective-wait doesn't head-of-line block the Sync/DVE queues
                # mid-kernel (engine queues are in-order).
                from concourse.tile_rust import add_dep_helper

                for k in range(N_CHUNK):
                    rb = op.tile([P, D], BF16, tag="rb", name="rb")
                    rb_dma = nc.sync.dma_start(rb[:], rs_out[k].ap())
                    add_dep_helper(
                        rb_dma.ins, last_osb_dma.ins, False, "readback after compute"
                    )
                    rb32 = op.tile([P, D], F32, tag="rb32", name="rb32")
                    nc.vector.tensor_copy(rb32[:], rb[:])
                    nc.sync.dma_start(out.ap()[k * P : (k + 1) * P, :], rb32[:])

    nc.compile()
    return nc


def _get_compiled():
    global _compiled
    if _compiled is None:
        _compiled = _build()
    return _compiled


def kernel(residual, W_Q, W_K, W_V, W_O):
    nc = _get_compiled()

    resid_t = np.ascontiguousarray(residual.reshape(SEQ, D).T.astype(np.float32)).astype(NP_BF16)
    # fold the 1/sqrt(DH) score scale into W_Q
    wq2 = (W_Q.astype(np.float64) / np.sqrt(DH)).reshape(NH * DH, D).astype(np.float32)
    wqt_full = np.ascontiguousarray(wq2.T)  # [D, NH*DH]
    wkt_full = np.ascontiguousarray(W_K.reshape(NKV * DH, D).T)  # [D, NKV*DH]
    wvt_full = np.ascontiguousarray(W_V.reshape(NKV * DH, D).T)
    wo_full = W_O.reshape(NH * DH, D)  # [NH*DH, D]

    mask_np = np.triu(np.ones((P, P), dtype=np.float32)).astype(NP_BF16)  # [k, q]: q>=k
    ident_np = np.eye(P, dtype=np.float32)

    in_maps = []
    for c in range(N_CORES):
        in_maps.append(
            {
                "resid_t": resid_t,
                "wqt": np.ascontiguousarray(
                    wqt_full[:, c * NHL : (c + 1) * NHL]
                ).astype(NP_BF16),
                "wkvt": np.ascontiguousarray(
                    np.concatenate(
                        [
                            wkt_full[:, c * DH : (c + 1) * DH],
                            wvt_full[:, c * DH : (c + 1) * DH],
                        ],
                        axis=1,
                    )
                ).astype(NP_BF16),
                "wo": np.ascontiguousarray(
                    wo_full[c * NHL : (c + 1) * NHL, :]
                ).astype(NP_BF16),
                "mask": mask_np,
                "ident": ident_np,
            }
        )

    res = bass_utils.run_bass_kernel_spmd(
        nc, in_maps, core_ids=list(range(N_CORES))
    )
    kernel.last_results = res

    out_full = np.empty((SEQ, D), dtype=np.float32)
    for c in range(N_CORES):
        shard = res.results[c]["out"]  # [512, D]
        for k in range(N_CHUNK):
            out_full[1024 * k + P * c : 1024 * k + P * (c + 1)] = shard[
                k * P : (k + 1) * P
            ]
    return out_full.reshape(B, S, D)
